# revision 21
# baseline (speedup 1.0000x reference)
"""VQ codebook nearest-neighbor kernel for 8 TRN2 NeuronCores.

Problem: z (8, 2048, 512) f32, embedding (8192, 512) f32.
  flat = z.reshape(16384, 512)
  dist = |flat|^2 - 2 flat @ E^T + |E|^2          (16384, 8192)
  idx = argmin(dist, axis=1)
  z_quant = E[idx].reshape(8, 2048, 512)
  vq_loss = 2 * mean((z_quant - z)^2)

Sharding: data-parallel over the batch dim B=8 -> one batch of 2048 tokens
per core; the codebook is replicated. Outputs are assembled on the host.

Per-core device algorithm (tokens on partitions, codes on the free axis):
  s = 2*z@E^T - |E|^2  ==  |z|^2 - dist;  argmax_k s == argmin_k dist,
  and min-dist = |z|^2 - max_k s (per-token |z|^2 is a host-side input,
  so the loss needs no extra device pass).

  Precision: plain fp32 matmuls run at 4 cycles/row on the TRN2 PE. Instead
  the dot products use a compensated bf16 3-term scheme at 1 cycle/row:
  z ~ zh + zl and 2E ~ eh + el (bf16 hi/lo pairs, ~17 mantissa bits each);
  s = zh@eh + zh@el + zl@eh accumulated in fp32 PSUM. Residual error
  ~1e-4, small vs the typical top-1/top-2 distance gap (~10), and measured
  bit-exact argmin agreement with the fp32 reference on the eval seed.
  The -|E|^2 bias is folded into the same PSUM accumulation as one K=2
  rank-2 matmul: ones(2,128)^T @ [nen_hi; nen_lo], where nen_hi/lo is an
  exact float32r (11-bit RNE) hi/lo split so the bias carries ~24 bits.
  The hi/lo rows are packed at partition bases {0,32} of one SBUF tile
  (matmul operands must sit at base partition 0/32/64).

  Engines:
  - PE: per (128-token group, 512-code tile): 12 bf16 matmuls + 1 f32r
    rank-2 matmul, all accumulating in one PSUM bank.
  - ACT: copies each PSUM tile into an SBUF distance half-row [128, 4096].
  - DVE: per half-row: nc.vector.max (top-8) + nc.vector.max_index (first
    occurrence -> matches jnp.argmin tie-break). Halves combined with a few
    [128,1] ALU ops (strict > keeps the lower half on ties). Loss column =
    znorm - max(s).
  - GPSIMD: indirect DMA gathers embedding[idx] rows into SBUF; plain DMA
    writes them to the output.
"""

import ml_dtypes
import numpy as np

import concourse.bass as bass
import concourse.mybir as mybir
import concourse.tile as tile
from concourse import bacc
from concourse.bass import IndirectOffsetOnAxis
from concourse.bass_utils import run_bass_kernel_spmd

F32 = mybir.dt.float32
F32R = mybir.dt.float32r
BF16 = mybir.dt.bfloat16
P = 128  # partitions / tokens per group
NTILE = 512  # codes per psum tile (fp32 moving-operand max, one PSUM bank)

B, L, D, K = 8, 2048, 512, 8192
N_CORES = 8


def _round_f32r(x):
    """Round-to-nearest-even at 11 explicit mantissa bits (the PE's f32r
    ingest rounding, measured on HW)."""
    u = np.ascontiguousarray(x, dtype=np.float32).view(np.uint32)
    shift = 12
    lsb = ((u >> np.uint32(shift)) & np.uint32(1)).astype(np.uint32)
    r = ((u + np.uint32((1 << (shift - 1)) - 1) + lsb) >> np.uint32(shift)) << np.uint32(shift)
    return r.view(np.float32)


def build(L_core=L, Kc=K, Dc=D):
    """Build the single-core Bass program (same NEFF runs SPMD on all cores)."""
    nc = bacc.Bacc(
        "TRN2",
        target_bir_lowering=False,
        debug=False,
        num_devices=N_CORES,
        dynamic_dma_scratch_size=1 << 15,
    )

    G = L_core // P  # token groups
    NC_CH = Dc // P  # contraction chunks (4)
    NT = Kc // NTILE  # code tiles (16)
    HALF_T = NT // 2  # tiles per half (8)
    HALF_W = HALF_T * NTILE  # codes per half (4096)

    zT_h = nc.dram_tensor("zT_h", [Dc, L_core], BF16, kind="ExternalInput")
    zT_l = nc.dram_tensor("zT_l", [Dc, L_core], BF16, kind="ExternalInput")
    znorm = nc.dram_tensor("znorm", [L_core, 1], F32, kind="ExternalInput")
    e2t_h = nc.dram_tensor("e2t_h", [Dc, Kc], BF16, kind="ExternalInput")
    e2t_l = nc.dram_tensor("e2t_l", [Dc, Kc], BF16, kind="ExternalInput")
    nenorm_hl = nc.dram_tensor("nenorm_hl", [2, Kc], F32R, kind="ExternalInput")
    etab = nc.dram_tensor("etab", [Kc, Dc], F32, kind="ExternalInput")

    zq_out = nc.dram_tensor("zq", [L_core, Dc], F32, kind="ExternalOutput")
    lp_out = nc.dram_tensor("loss_parts", [P, G], F32, kind="ExternalOutput")

    with tile.TileContext(nc) as tc:
        with (
            tc.tile_pool(name="const", bufs=1) as constp,
            tc.tile_pool(name="dist", bufs=1) as distp,
            tc.tile_pool(name="work", bufs=2) as workp,
            tc.tile_pool(name="small", bufs=2) as smallp,
            tc.tile_pool(name="psum", bufs=8, space="PSUM") as psump,
        ):
            # --- persistent constants -----------------------------------
            eth_sb = []
            etl_sb = []
            for c in range(NC_CH):
                th = constp.tile([P, Kc], BF16, tag=f"eth{c}", name=f"eth{c}")
                eth_sb.append(th)
                tl = constp.tile([P, Kc], BF16, tag=f"etl{c}", name=f"etl{c}")
                etl_sb.append(tl)
            # quarter-sliced loads: first tiles' matmuls start once the
            # first quarter lands instead of waiting for whole chunks
            QW = Kc // 4
            for q in range(4):
                qsl = slice(q * QW, (q + 1) * QW)
                for c in range(NC_CH):
                    nc.sync.dma_start(
                        out=eth_sb[c][:, qsl], in_=e2t_h[c * P : (c + 1) * P, qsl]
                    )
                    nc.sync.dma_start(
                        out=etl_sb[c][:, qsl], in_=e2t_l[c * P : (c + 1) * P, qsl]
                    )
            ones_f = constp.tile([34, P], F32, tag="ones_f", name="ones_f")
            nc.vector.memset(ones_f[:], 1.0)
            ones_sb = constp.tile([34, P], F32R, tag="ones", name="ones")
            nc.vector.tensor_copy(out=ones_sb[:], in_=ones_f[:])
            nen_sb = constp.tile([34, HALF_W], F32R, tag="nen", name="nen")
            nc.sync.dma_start(out=nen_sb[0:2, :], in_=nenorm_hl[:, 0:HALF_W])
            nc.sync.dma_start(out=nen_sb[32:34, :], in_=nenorm_hl[:, HALF_W:Kc])
            lp_sb = constp.tile([P, G], F32, tag="lp", name="lp")

            for g in range(G):
                gsl = slice(g * P, (g + 1) * P)
                # z^T chunk block [k, c, m] for this token group
                zth_g = workp.tile([P, NC_CH, P], BF16, tag="zth", name="zth")
                nc.sync.dma_start(
                    out=zth_g[:],
                    in_=zT_h[:, gsl].rearrange("(c k) m -> k c m", k=P),
                )
                ztl_g = workp.tile([P, NC_CH, P], BF16, tag="ztl", name="ztl")
                nc.sync.dma_start(
                    out=ztl_g[:],
                    in_=zT_l[:, gsl].rearrange("(c k) m -> k c m", k=P),
                )
                zn_g = workp.tile([P, 1], F32, tag="zn", name="zn")
                nc.sync.dma_start(out=zn_g[:], in_=znorm[gsl, :])

                halves = []
                for h in range(2):
                    dh = distp.tile([P, HALF_W], F32, tag=f"dh{h}", name=f"dh{h}")
                    for j in range(HALF_T):
                        n = h * HALF_T + j
                        nsl = slice(n * NTILE, (n + 1) * NTILE)
                        jsl = slice(j * NTILE, (j + 1) * NTILE)
                        ps = psump.tile([P, NTILE], F32, tag="ps", name="ps")
                        for c in range(NC_CH):
                            nc.tensor.matmul(
                                ps[:],
                                zth_g[:, c, :],
                                eth_sb[c][:, nsl],
                                start=(c == 0),
                                stop=False,
                            )
                        for c in range(NC_CH):
                            nc.tensor.matmul(
                                ps[:],
                                zth_g[:, c, :],
                                etl_sb[c][:, nsl],
                                start=False,
                                stop=False,
                            )
                        for c in range(NC_CH):
                            nc.tensor.matmul(
                                ps[:],
                                ztl_g[:, c, :],
                                eth_sb[c][:, nsl],
                                start=False,
                                stop=False,
                            )
                        # fold -|E|^2: K=2 rank-2 update (hi+lo rows packed
                        # at partition base 32*h of nen_sb)
                        nb = 32 * h
                        nc.tensor.matmul(
                            ps[:],
                            ones_sb[nb : nb + 2, :],
                            nen_sb[nb : nb + 2, jsl],
                            start=False,
                            stop=True,
                        )
                        # PSUM -> SBUF dist half (ACT engine)
                        nc.scalar.activation(
                            out=dh[:, jsl],
                            in_=ps[:],
                            func=mybir.ActivationFunctionType.Copy,
                        )
                    top8 = smallp.tile(
                        [P, 8], F32, tag=f"top8_{h}", name=f"top8_{h}"
                    )
                    nc.vector.max(out=top8[:], in_=dh[:])
                    idx8 = smallp.tile(
                        [P, 8], mybir.dt.uint32, tag=f"idx8_{h}", name=f"idx8_{h}"
                    )
                    nc.vector.max_index(out=idx8[:], in_max=top8[:], in_values=dh[:])
                    halves.append((top8, idx8))

                (top8_0, idx8_0), (top8_1, idx8_1) = halves
                # combine halves: idx = idx0 if max0 >= max1 else HALF_W+idx1
                idxf0 = smallp.tile([P, 1], F32, tag="idxf0", name="idxf0")
                nc.vector.tensor_copy(out=idxf0[:], in_=idx8_0[:, 0:1])
                idxf1 = smallp.tile([P, 1], F32, tag="idxf1", name="idxf1")
                nc.vector.tensor_copy(out=idxf1[:], in_=idx8_1[:, 0:1])
                is1 = smallp.tile([P, 1], F32, tag="is1", name="is1")
                nc.vector.tensor_tensor(
                    out=is1[:],
                    in0=top8_1[:, 0:1],
                    in1=top8_0[:, 0:1],
                    op=mybir.AluOpType.is_gt,
                )
                mx = smallp.tile([P, 1], F32, tag="mx", name="mx")
                nc.vector.tensor_tensor(
                    out=mx[:],
                    in0=top8_1[:, 0:1],
                    in1=top8_0[:, 0:1],
                    op=mybir.AluOpType.max,
                )
                # loss column: min-dist = znorm - max(s)
                nc.vector.tensor_tensor(
                    out=lp_sb[:, g : g + 1],
                    in0=zn_g[:],
                    in1=mx[:],
                    op=mybir.AluOpType.subtract,
                )
                # dd = (idx1 + HALF_W) - idx0
                dd = smallp.tile([P, 1], F32, tag="dd", name="dd")
                nc.vector.scalar_tensor_tensor(
                    out=dd[:],
                    in0=idxf1[:],
                    scalar=float(HALF_W),
                    in1=idxf0[:],
                    op0=mybir.AluOpType.add,
                    op1=mybir.AluOpType.subtract,
                )
                # idxf = is1 * dd + idx0
                idxf = smallp.tile([P, 1], F32, tag="idxf", name="idxf")
                nc.vector.scalar_tensor_tensor(
                    out=idxf[:],
                    in0=is1[:],
                    scalar=dd[:, 0:1],
                    in1=idxf0[:],
                    op0=mybir.AluOpType.mult,
                    op1=mybir.AluOpType.add,
                )
                # clamp for gather safety (idx is < Kc by construction)
                nc.vector.tensor_scalar_min(idxf[:], idxf[:], float(Kc - 1))
                idxi = smallp.tile([P, 1], mybir.dt.int32, tag="idxi", name="idxi")
                nc.vector.tensor_copy(out=idxi[:], in_=idxf[:])

                # gather embedding rows
                zq_g = workp.tile([P, Dc], F32, tag="zq", name="zq")
                nc.gpsimd.indirect_dma_start(
                    out=zq_g[:],
                    out_offset=None,
                    in_=etab[:, :],
                    in_offset=IndirectOffsetOnAxis(ap=idxi[:, :1], axis=0),
                    bounds_check=Kc - 1,
                    oob_is_err=False,
                )
                nc.sync.dma_start(out=zq_out[gsl, :], in_=zq_g[:])

            nc.sync.dma_start(out=lp_out[:, :], in_=lp_sb[:])

    nc.finalize()
    return nc


_CACHE = {}


def _get_nc(L_core=L, Kc=K, Dc=D):
    key = (L_core, Kc, Dc)
    if key not in _CACHE:
        _CACHE[key] = build(L_core, Kc, Dc)
    return _CACHE[key]


def _run(z, embedding, trace=False, **run_kwargs):
    z = np.ascontiguousarray(np.asarray(z, dtype=np.float32))
    embedding = np.ascontiguousarray(np.asarray(embedding, dtype=np.float32))
    Bc, L_core, Dc = z.shape
    Kc = embedding.shape[0]

    nc = _get_nc(L_core, Kc, Dc)

    e2t = np.ascontiguousarray((2.0 * embedding).T)  # [D, K]
    e2t_h = e2t.astype(ml_dtypes.bfloat16)
    e2t_l = (e2t - e2t_h.astype(np.float32)).astype(ml_dtypes.bfloat16)
    nenorm = -np.einsum(
        "kd,kd->k", embedding, embedding, dtype=np.float32
    ).reshape(1, Kc)
    nenorm = np.ascontiguousarray(nenorm, dtype=np.float32)
    nen_h = _round_f32r(nenorm)
    nen_l = _round_f32r(nenorm - nen_h)
    nen_hl = np.ascontiguousarray(np.concatenate([nen_h, nen_l], axis=0))

    in_maps = []
    for c in range(Bc):
        zc = z[c]
        zct = np.ascontiguousarray(zc.T)
        zt_h = zct.astype(ml_dtypes.bfloat16)
        zt_l = (zct - zt_h.astype(np.float32)).astype(ml_dtypes.bfloat16)
        zn = np.einsum("ld,ld->l", zc, zc, dtype=np.float32).reshape(L_core, 1)
        in_maps.append(
            {
                "zT_h": zt_h,
                "zT_l": zt_l,
                "znorm": np.ascontiguousarray(zn),
                "e2t_h": e2t_h,
                "e2t_l": e2t_l,
                "nenorm_hl": nen_hl,
                "etab": embedding,
            }
        )

    res = run_bass_kernel_spmd(
        nc, in_maps, core_ids=list(range(Bc)), trace=trace, **run_kwargs
    )

    zq = np.stack([r["zq"] for r in res.results], axis=0)  # (B, L, D)
    total = np.float64(0.0)
    for r in res.results:
        total += r["loss_parts"].astype(np.float64).sum()
    vq_loss = np.float32(2.0 * total / (Bc * L_core * Dc))
    return (zq, vq_loss), res


def kernel(z, embedding):
    (zq, vq_loss), _ = _run(z, embedding)
    return (zq, vq_loss)
